# revision 7
# baseline (speedup 1.0000x reference)
"""Fused GPTQ-style dequant + GEMM kernel for 8 TRN2 NeuronCores.

Reference computation (per problem):
    w = (q - zp[g]) * scale[g]   per group g of 128 consecutive k values
    out = active @ w + bias      active [256, 4096], w [4096, 11008]

Sharding: tensor-parallel along N (output features). Each of 8 cores gets
an 11008/8 = 1376-wide slice of weight/scale/zp/bias; activations are
replicated; outputs concatenated on host.

Device algorithm (per core):
    out = aT.T @ (q * scale_bcast)  +  (-r).T @ (zp*scale)  +  1 x bias
  - weights shipped as bf16 codes (0..15 exact), partition-major for big
    DMA descriptors; dequantized on VectorE as q*scale.
  - scale broadcast to all 128 partitions: one SWDGE f32->bf16 cast into a
    DRAM staging row, then per scale-tile one HWDGE stride-0 replicated
    seed read (32 partitions) and ONE fanout DMA (source read 3x via a
    stride-0 repeat dim). DMA instruction count is the scarce resource.
  - zero-point term folds into a rank-32 correction GEMM via per-group
    activation row-sums r (TensorE, -onehot blocks embedded in the weight
    tensor's trailing columns); bias rides the correction GEMM as row 32
    (zp/bias/scale/ones ship as one fused "meta" input).
  - All 32 group matmuls + correction accumulate in the same PSUM banks.
"""

import sys

sys.path.insert(0, "/opt/trn_rl_repo")

import numpy as np
import ml_dtypes

import concourse.bass as bass
import concourse.bacc as bacc
import concourse.mybir as mybir
import concourse.tile as tile
from concourse.bass import ts, ds

BF16 = mybir.dt.bfloat16
F32 = mybir.dt.float32

P = 128           # partitions / group size
G = 32            # quant groups
K = 4096          # contraction dim
S = 256           # sequence (rows of activation)
N_FULL = 11008
NCORES = 8
NSH = N_FULL // NCORES      # 1376 output features per core
NW = NSH + G                # weight row width incl. -onehot block
GCHUNK = 4                  # weight groups per DMA/dequant chunk
NCHUNKS = G // GCHUNK
SCCH = 8                    # groups per scale-broadcast tile
ATCH = 16                   # groups per activation slice-tile
N_SPLITS = (512, 512, 352)  # psum free-dim chunking of NSH

_NC_CACHE = {}


def build_nc():
    """Build the single-core Bass graph (same graph runs SPMD on all 8 cores)."""
    nc = bacc.Bacc(None)

    aT_d = nc.declare_dram_parameter("aT", [P, G, S], F32, isOutput=False)
    wq_d = nc.declare_dram_parameter("wq", [P, G, NW], BF16, isOutput=False)
    # meta[:,0,:] rows = (zp[0..31]; bias), meta[:,1,:] rows = (scale; ones)
    meta_d = nc.declare_dram_parameter("meta", [G + 1, 2, NSH], F32, isOutput=False)
    out_d = nc.declare_dram_parameter("out", [S, NSH], F32, isOutput=True)

    n_off = [0, 512, 1024]

    with tile.TileContext(nc) as tc:
        with (
            tc.tile_pool(name="const", bufs=1) as const,
            tc.tile_pool(name="wpool", bufs=3) as wpool,
            tc.tile_pool(name="psum", bufs=1, space="PSUM") as psum,
        ):
            # ---------------- preamble ----------------
            # stage scale as bf16 in DRAM (one SWDGE cast) so seed DMAs are
            # cast-free HWDGE reads
            sc_bf_d = nc.dram_tensor("sc_bf_stage", [G, NSH], BF16, kind="Internal")
            nc.gpsimd.dma_start(sc_bf_d[:], meta_d[0:G, 1, :])

            # activations: partition-major f32, SWDGE cast to bf16, 2 halves
            aT = []
            for q in range(G // ATCH):
                t = const.tile([P, ATCH, S], BF16, name=f"aT{q}")
                nc.gpsimd.dma_start(t[:], aT_d[:, ts(q, ATCH), :])
                aT.append(t)

            meta = const.tile([G + 1, 2, NSH], F32)
            nc.sync.dma_start(meta[:], meta_d[:])

            # correction rhs rows 0..31 = zp*scale, row 32 = bias*1, rest 0
            corr_rhs = const.tile([64, NSH], BF16)
            nc.vector.memset(corr_rhs[:], 0.0)
            nc.vector.tensor_tensor(
                corr_rhs[0 : G + 1, :], meta[:, 0, :], meta[:, 1, :],
                mybir.AluOpType.mult,
            )

            # scale broadcast tiles (2 weight-chunks each): stride-0 seed +
            # one repeat-3 fanout, both HWDGE, no dependency chains
            scb = [
                const.tile([P, SCCH, NSH], BF16, name=f"scb{t}")
                for t in range(G // SCCH)
            ]
            for t in range(G // SCCH):
                src = sc_bf_d[None, ts(t, SCCH), :].to_broadcast([32, SCCH, NSH])
                nc.sync.dma_start(scb[t][0:32], src)
            for t in range(G // SCCH):
                ring = nc.sync if t % 2 == 0 else nc.scalar
                src3 = scb[t][0:32][:, None, :, :].to_broadcast([32, 3, SCCH, NSH])
                ring.dma_start(scb[t][32:128], src3)

            # psum accumulators: [2 s-chunks][3 n-chunks] + r accumulator
            acc = [
                [psum.tile([P, nw], F32, name=f"acc_{si}_{nj}") for nj, nw in enumerate(N_SPLITS)]
                for si in range(2)
            ]
            psum_r = psum.tile([G, S], F32, name="psum_r")

            # ---------------- main loop ----------------
            for c in range(NCHUNKS):
                g0 = c * GCHUNK
                wq = wpool.tile([P, GCHUNK, NW], BF16, tag="wq")
                nc.scalar.dma_start(wq[:], wq_d[:, ts(c, GCHUNK), :])
                # dequant in place: w *= scale (partition-broadcast tile);
                # the trailing -onehot block stays unscaled
                nc.vector.tensor_tensor(
                    wq[:, :, 0:NSH],
                    wq[:, :, 0:NSH],
                    scb[g0 // SCCH][:, ds((g0 % SCCH), GCHUNK), :],
                    mybir.AluOpType.mult,
                )
                for gl in range(GCHUNK):
                    g = g0 + gl
                    a_g = aT[g // ATCH][:, g % ATCH, :]
                    # r accumulation via the embedded -onehot block:
                    # psum_r[j,s] += sum_p -1[j==g] * aT[p,g,s]
                    nc.tensor.matmul(
                        psum_r[:],
                        wq[:, gl, NSH:NW],
                        a_g,
                        start=(g == 0),
                        stop=(g == G - 1),
                    )
                    for si in range(2):
                        lhsT = a_g[:, ts(si, P)]
                        for nj, nw in enumerate(N_SPLITS):
                            nc.tensor.matmul(
                                acc[si][nj][:, :nw],
                                lhsT,
                                wq[:, gl, ds(n_off[nj], nw)],
                                start=(g == 0),
                                stop=False,
                            )

            # ---------------- correction + epilogue ----------------
            corr_lhsT = const.tile([64, S], BF16)
            nc.vector.memset(corr_lhsT[:], 0.0)
            nc.vector.tensor_copy(corr_lhsT[0:G, :], psum_r[:])
            nc.vector.memset(corr_lhsT[G : G + 1, :], 1.0)

            out_sb = const.tile([P, 2, NSH], F32)   # 11 KB/part
            for si in range(2):
                for nj, nw in enumerate(N_SPLITS):
                    nc.tensor.matmul(
                        acc[si][nj][:, :nw],
                        corr_lhsT[:, ts(si, P)],
                        corr_rhs[:, ds(n_off[nj], nw)],
                        start=False,
                        stop=True,
                    )
                    if (si + nj) % 2:
                        nc.scalar.copy(
                            out_sb[:, si, ds(n_off[nj], nw)], acc[si][nj][:, :nw]
                        )
                    else:
                        nc.vector.tensor_copy(
                            out_sb[:, si, ds(n_off[nj], nw)], acc[si][nj][:, :nw]
                        )

            nc.sync.dma_start(out_d.rearrange("(so p) n -> p so n", p=P), out_sb[:])

    nc.compile()
    return nc


def _prep_in_maps(active, weight, scale, zp, bias):
    a2 = np.asarray(active, dtype=np.float32).reshape(S, K)
    # aT partition-major: [P, G, S] where k = g*128 + p
    aTp = np.ascontiguousarray(a2.T.reshape(G, P, S).transpose(1, 0, 2))
    wq_bf = np.asarray(weight).astype(ml_dtypes.bfloat16)  # codes 0..15, exact
    scale = np.asarray(scale, dtype=np.float32)
    zp = np.asarray(zp, dtype=np.float32)
    bias = np.asarray(bias, dtype=np.float32)

    # -onehot blocks appended to each group's weight rows
    negI = np.broadcast_to(
        -np.eye(G, dtype=ml_dtypes.bfloat16)[None, :, :], (P, G, G)
    )

    in_maps = []
    for i in range(NCORES):
        sl = slice(i * NSH, (i + 1) * NSH)
        wq = np.empty((P, G, NW), dtype=ml_dtypes.bfloat16)
        wq[:, :, 0:NSH] = wq_bf[:, :, sl].transpose(1, 0, 2)
        wq[:, :, NSH:NW] = negI
        meta = np.empty((G + 1, 2, NSH), dtype=np.float32)
        meta[0:G, 0, :] = zp[:, sl]
        meta[G, 0, :] = bias[sl]
        meta[0:G, 1, :] = scale[:, sl]
        meta[G, 1, :] = 1.0
        in_maps.append(
            {
                "aT": aTp,
                "wq": np.ascontiguousarray(wq),
                "meta": meta,
            }
        )
    return in_maps


def run_on_hw(inputs, trace=False):
    """Run the SPMD kernel; returns (full_output, BassKernelResults)."""
    from concourse.bass_utils import run_bass_kernel_spmd

    if "nc" not in _NC_CACHE:
        _NC_CACHE["nc"] = build_nc()
    nc = _NC_CACHE["nc"]
    in_maps = _prep_in_maps(
        inputs["active"], inputs["weight"], inputs["scale"],
        inputs["zp"], inputs["bias"],
    )
    res = run_bass_kernel_spmd(
        nc, in_maps, core_ids=list(range(NCORES)), trace=trace
    )
    parts = [np.asarray(res.results[i]["out"]) for i in range(NCORES)]
    full = np.concatenate(parts, axis=-1).reshape(1, 1, S, N_FULL)
    return np.ascontiguousarray(full, dtype=np.float32), res


def kernel(**inputs) -> np.ndarray:
    assert int(inputs.get("group_size", P)) == P
    assert int(inputs.get("weight_bits", 4)) == 4
    out, _ = run_on_hw(inputs, trace=False)
    return out


# revision 10
# speedup vs baseline: 1.0359x; 1.0359x over previous
"""Fused GPTQ-style dequant + GEMM kernel for 8 TRN2 NeuronCores.

Reference computation (per problem):
    w = (q - zp[g]) * scale[g]   per group g of 128 consecutive k values
    out = active @ w + bias      active [256, 4096], w [4096, 11008]

Sharding: tensor-parallel along N (output features). Each of 8 cores gets
an 11008/8 = 1376-wide slice of weight/scale/zp/bias; activations are
replicated; outputs concatenated on host.

Device algorithm (per core):
    out = aT.T @ (q * scale_bcast)  +  (-r).T @ (zp*scale)  +  1 x bias
  - weights shipped as bf16 codes (0..15 exact), partition-major for big
    DMA descriptors; dequantized on VectorE as q*scale.
  - scale broadcast to all 128 partitions: one SWDGE f32->bf16 cast into a
    DRAM staging row, then per scale-tile one HWDGE stride-0 replicated
    seed read (32 partitions) and ONE fanout DMA (source read 3x via a
    stride-0 repeat dim). DMA instruction count is the scarce resource.
  - zero-point term folds into a rank-32 correction GEMM via per-group
    activation row-sums r (TensorE, -onehot blocks embedded in the weight
    tensor's trailing columns); bias rides the correction GEMM as row 32
    (zp/bias/scale/ones ship as one fused "meta" input).
  - All 32 group matmuls + correction accumulate in the same PSUM banks.
"""

import sys

sys.path.insert(0, "/opt/trn_rl_repo")

import numpy as np
import ml_dtypes

import concourse.bass as bass
import concourse.bacc as bacc
import concourse.mybir as mybir
import concourse.tile as tile
from concourse.bass import ts, ds

BF16 = mybir.dt.bfloat16
F32 = mybir.dt.float32

P = 128           # partitions / group size
G = 32            # quant groups
K = 4096          # contraction dim
S = 256           # sequence (rows of activation)
N_FULL = 11008
NCORES = 8
NSH = N_FULL // NCORES      # 1376 output features per core
NW = NSH + G                # weight row width incl. -onehot block
GCHUNK = 4                  # weight groups per DMA/dequant chunk
NCHUNKS = G // GCHUNK
SCCH = 8                    # groups per scale-broadcast tile
ATCH = 16                   # groups per activation slice-tile
N_SPLITS = (512, 512, 352)  # psum free-dim chunking of NSH

_NC_CACHE = {}


def build_nc():
    """Build the single-core Bass graph (same graph runs SPMD on all 8 cores)."""
    nc = bacc.Bacc(None)

    aT_d = nc.declare_dram_parameter("aT", [P, G, S], F32, isOutput=False)
    wq_d = nc.declare_dram_parameter("wq", [P, G, NW], BF16, isOutput=False)
    # meta[:,0,:] rows = (zp[0..31]; bias), meta[:,1,:] rows = (scale; ones)
    meta_d = nc.declare_dram_parameter("meta", [G + 1, 2, NSH], F32, isOutput=False)
    scbf_d = nc.declare_dram_parameter("scbf", [G, NSH], BF16, isOutput=False)
    out_d = nc.declare_dram_parameter("out", [S, NSH], F32, isOutput=True)

    n_off = [0, 512, 1024]

    with tile.TileContext(nc) as tc:
        with (
            tc.tile_pool(name="const", bufs=1) as const,
            tc.tile_pool(name="wpool", bufs=4) as wpool,
            tc.tile_pool(name="psum", bufs=1, space="PSUM") as psum,
        ):
            # ---------------- preamble ----------------
            # scale broadcast tiles (2 weight-chunks each): one HWDGE
            # stride-0 replicated seed read from the bf16 scale input, then
            # repeat-read fanouts. Even tiles seed partitions [0:32] (even
            # DMA engines), odd tiles [64:96] (odd engines) so concurrent
            # fanout source reads spread across all 16 engines.
            scb = [
                const.tile([P, SCCH, NSH], BF16, name=f"scb{t}")
                for t in range(G // SCCH)
            ]
            for t in range(G // SCCH):
                src = scbf_d[None, ts(t, SCCH), :].to_broadcast([32, SCCH, NSH])
                if t % 2 == 0:
                    nc.sync.dma_start(scb[t][0:32], src)
                else:
                    nc.sync.dma_start(scb[t][64:96], src)
            for t in range(G // SCCH):
                seed = scb[t][0:32] if t % 2 == 0 else scb[t][64:96]
                if t % 2 == 0:
                    src3 = seed[:, None, :, :].to_broadcast([32, 3, SCCH, NSH])
                    nc.sync.dma_start(scb[t][32:128], src3)
                else:
                    src2 = seed[:, None, :, :].to_broadcast([32, 2, SCCH, NSH])
                    nc.gpsimd.dma_start(scb[t][0:64], src2)
                    nc.gpsimd.dma_start(scb[t][96:128], seed)

            # activations: partition-major f32, SWDGE cast to bf16, 2 halves
            aT = []
            for q in range(G // ATCH):
                t = const.tile([P, ATCH, S], BF16, name=f"aT{q}")
                nc.gpsimd.dma_start(t[:], aT_d[:, ts(q, ATCH), :])
                aT.append(t)

            meta = const.tile([G + 1, 2, NSH], F32)
            nc.sync.dma_start(meta[:], meta_d[:])

            # correction rhs rows 0..31 = zp*scale, row 32 = bias*1, rest 0
            corr_rhs = const.tile([64, NSH], BF16)
            nc.vector.memset(corr_rhs[:], 0.0)
            nc.vector.tensor_tensor(
                corr_rhs[0 : G + 1, :], meta[:, 0, :], meta[:, 1, :],
                mybir.AluOpType.mult,
            )

            # psum accumulators: [2 s-chunks][3 n-chunks] + r accumulator
            acc = [
                [psum.tile([P, nw], F32, name=f"acc_{si}_{nj}") for nj, nw in enumerate(N_SPLITS)]
                for si in range(2)
            ]
            psum_r = psum.tile([G, S], F32, name="psum_r")

            # ---------------- main loop ----------------
            for c in range(NCHUNKS):
                g0 = c * GCHUNK
                wq = wpool.tile([P, GCHUNK, NW], BF16, tag="wq")
                nc.scalar.dma_start(wq[:], wq_d[:, ts(c, GCHUNK), :])
                # dequant in place: w *= scale (partition-broadcast tile);
                # the trailing -onehot block stays unscaled
                nc.vector.tensor_tensor(
                    wq[:, :, 0:NSH],
                    wq[:, :, 0:NSH],
                    scb[g0 // SCCH][:, ds((g0 % SCCH), GCHUNK), :],
                    mybir.AluOpType.mult,
                )
                for gl in range(GCHUNK):
                    g = g0 + gl
                    a_g = aT[g // ATCH][:, g % ATCH, :]
                    # r accumulation via the embedded -onehot block:
                    # psum_r[j,s] += sum_p -1[j==g] * aT[p,g,s]
                    nc.tensor.matmul(
                        psum_r[:],
                        wq[:, gl, NSH:NW],
                        a_g,
                        start=(g == 0),
                        stop=(g == G - 1),
                    )
                    for si in range(2):
                        lhsT = a_g[:, ts(si, P)]
                        for nj, nw in enumerate(N_SPLITS):
                            nc.tensor.matmul(
                                acc[si][nj][:, :nw],
                                lhsT,
                                wq[:, gl, ds(n_off[nj], nw)],
                                start=(g == 0),
                                stop=False,
                            )

            # ---------------- correction + epilogue ----------------
            corr_lhsT = const.tile([64, S], BF16)
            nc.vector.memset(corr_lhsT[:], 0.0)
            nc.vector.tensor_copy(corr_lhsT[0:G, :], psum_r[:])
            nc.vector.memset(corr_lhsT[G : G + 1, :], 1.0)

            out_sb = const.tile([P, 2, NSH], F32)   # 11 KB/part
            for si in range(2):
                for nj, nw in enumerate(N_SPLITS):
                    nc.tensor.matmul(
                        acc[si][nj][:, :nw],
                        corr_lhsT[:, ts(si, P)],
                        corr_rhs[:, ds(n_off[nj], nw)],
                        start=False,
                        stop=True,
                    )
                    if (si + nj) % 2:
                        nc.scalar.copy(
                            out_sb[:, si, ds(n_off[nj], nw)], acc[si][nj][:, :nw]
                        )
                    else:
                        nc.vector.tensor_copy(
                            out_sb[:, si, ds(n_off[nj], nw)], acc[si][nj][:, :nw]
                        )

            nc.sync.dma_start(out_d.rearrange("(so p) n -> p so n", p=P), out_sb[:])

    nc.compile()
    return nc


def _prep_in_maps(active, weight, scale, zp, bias):
    a2 = np.asarray(active, dtype=np.float32).reshape(S, K)
    # aT partition-major: [P, G, S] where k = g*128 + p
    aTp = np.ascontiguousarray(a2.T.reshape(G, P, S).transpose(1, 0, 2))
    wq_bf = np.asarray(weight).astype(ml_dtypes.bfloat16)  # codes 0..15, exact
    scale = np.asarray(scale, dtype=np.float32)
    zp = np.asarray(zp, dtype=np.float32)
    bias = np.asarray(bias, dtype=np.float32)

    # -onehot blocks appended to each group's weight rows
    negI = np.broadcast_to(
        -np.eye(G, dtype=ml_dtypes.bfloat16)[None, :, :], (P, G, G)
    )

    in_maps = []
    for i in range(NCORES):
        sl = slice(i * NSH, (i + 1) * NSH)
        wq = np.empty((P, G, NW), dtype=ml_dtypes.bfloat16)
        wq[:, :, 0:NSH] = wq_bf[:, :, sl].transpose(1, 0, 2)
        wq[:, :, NSH:NW] = negI
        meta = np.empty((G + 1, 2, NSH), dtype=np.float32)
        meta[0:G, 0, :] = zp[:, sl]
        meta[G, 0, :] = bias[sl]
        meta[0:G, 1, :] = scale[:, sl]
        meta[G, 1, :] = 1.0
        in_maps.append(
            {
                "aT": aTp,
                "wq": np.ascontiguousarray(wq),
                "meta": meta,
                "scbf": np.ascontiguousarray(scale[:, sl].astype(ml_dtypes.bfloat16)),
            }
        )
    return in_maps


def run_on_hw(inputs, trace=False):
    """Run the SPMD kernel; returns (full_output, BassKernelResults)."""
    from concourse.bass_utils import run_bass_kernel_spmd

    if "nc" not in _NC_CACHE:
        _NC_CACHE["nc"] = build_nc()
    nc = _NC_CACHE["nc"]
    in_maps = _prep_in_maps(
        inputs["active"], inputs["weight"], inputs["scale"],
        inputs["zp"], inputs["bias"],
    )
    res = run_bass_kernel_spmd(
        nc, in_maps, core_ids=list(range(NCORES)), trace=trace
    )
    parts = [np.asarray(res.results[i]["out"]) for i in range(NCORES)]
    full = np.concatenate(parts, axis=-1).reshape(1, 1, S, N_FULL)
    return np.ascontiguousarray(full, dtype=np.float32), res


def kernel(**inputs) -> np.ndarray:
    assert int(inputs.get("group_size", P)) == P
    assert int(inputs.get("weight_bits", 4)) == 4
    out, _ = run_on_hw(inputs, trace=False)
    return out
